# revision 5
# baseline (speedup 1.0000x reference)
# Trainium2 Bass kernel for per-sample channel-attention module (CAM).
#
# Reference math per sample (C=512, N=H*W=4096):
#   X = x.reshape(C, N)
#   phi = Wp X ; theta = Wt X ; g = Wg X
#   attn = softmax_rows(phi @ theta^T)          # [C, C]
#   y = attn @ g                                 # [C, N]
#   Z = (y^T).flatten().reshape(C, N)            # torch permute+view reinterpretation
#   out = gamma * (Wm @ Z) + x
#
# Algebraic restructuring used here (cuts PE work ~1.8x):
#   G = X X^T                  (Gram, [C, C])
#   L = Wp G Wt^T              (attention logits via two small GEMMs)
#   A' = softmax(L) @ Wg       (fold g-projection into attention)
#   y = A' X                   (single big GEMM)
# The Z reinterpretation is handled for free by computing y^T blocks with a
# stride-8 column selection of X as the stationary matmul operand, so each
# PSUM tile lands exactly on a contiguous block of Z's SBUF layout.
#
# All matmuls/transposes run in float32r (fp32 data, PE streams it at
# 1 row/cycle; values carry 11 explicit mantissa bits). The BIR verifier
# requires every fp32r matmul operand to be produced as fp32r, so operand
# tiles are allocated float32r and the host pre-rounds the input tensors
# to the fp32r grid (round-to-nearest-even, 11 mantissa bits).

import os
import numpy as np

import concourse.bass as bass
import concourse.mybir as mybir
import concourse.tile as tile
from concourse import bacc
from concourse.bass_utils import run_bass_kernel_spmd
from concourse.tile import TileContext
from concourse.masks import make_identity

P = 128          # partitions
C = 512          # channels
N = 4096         # spatial (64*64)
CC = C // P      # 4 channel chunks
NT = N // P      # 32 spatial tiles
QF = N // C      # 8 fold factor for the permute+view reinterpretation
FP32 = mybir.dt.float32
FP32R = mybir.dt.float32r


def _f32(ap):
    # reinterpret an fp32r tile as plain fp32 (identical bit layout)
    return ap.bitcast(FP32)


def _build_nc():
    nc = bacc.Bacc("TRN2", target_bir_lowering=False, debug=False, num_devices=8)
    # x is consumed both as a matmul operand and in the residual add, so it
    # is declared fp32r and pre-rounded host-side (residual error ~2^-13).
    x_d = nc.dram_tensor("x", [C, N], FP32R, kind="ExternalInput").ap()
    wphi_d = nc.dram_tensor("w_phi", [C, C], FP32, kind="ExternalInput").ap()
    wtheta_d = nc.dram_tensor("w_theta", [C, C], FP32, kind="ExternalInput").ap()
    wg_d = nc.dram_tensor("w_g", [C, C], FP32R, kind="ExternalInput").ap()
    wmask_d = nc.dram_tensor("w_mask", [C, C], FP32, kind="ExternalInput").ap()
    gamma_d = nc.dram_tensor("gamma", [1], FP32, kind="ExternalInput").ap()
    out_d = nc.dram_tensor("out", [C, N], FP32, kind="ExternalOutput").ap()

    with TileContext(nc) as tc:
        _body(tc, x_d, wphi_d, wtheta_d, wg_d, wmask_d, gamma_d, out_d)
    nc.compile()
    return nc


def _body(tc, x_d, wphi_d, wtheta_d, wg_d, wmask_d, gamma_d, out_d):
    nc = tc.nc
    from contextlib import ExitStack

    with ExitStack() as ctx:
        const = ctx.enter_context(tc.tile_pool(name="const", bufs=1))
        xpool = ctx.enter_context(tc.tile_pool(name="xpool", bufs=1))
        wpool = ctx.enter_context(tc.tile_pool(name="wpool", bufs=1))
        bigpool = ctx.enter_context(tc.tile_pool(name="bigpool", bufs=1))
        scratch = ctx.enter_context(tc.tile_pool(name="scratch", bufs=2))
        vecs = ctx.enter_context(tc.tile_pool(name="vecs", bufs=8))
        outp = ctx.enter_context(tc.tile_pool(name="outp", bufs=4))
        ps = ctx.enter_context(tc.tile_pool(name="ps", bufs=6, space="PSUM"))

        # Transposes run in plain fp32 (exact); the PSUM->SBUF copy after
        # each transpose performs the fp32r rounding for matmul operands.
        identity = const.tile([P, P], FP32)
        make_identity(nc, identity)

        gamma_t = const.tile([P, 1], FP32)
        nc.gpsimd.dma_start(out=gamma_t, in_=gamma_d.to_broadcast([P, 1]))

        # ---- load weights (natural layout, [p, cc, j] <=> W[128*cc+p, j]) and
        # build transposed copies WT[p, jc, d] = W[d, 128*jc + p].
        wg_sb = wpool.tile([P, CC, C], FP32R)
        nc.sync.dma_start(out=wg_sb, in_=wg_d.rearrange("(cc p) j -> p cc j", p=P))

        wphiT = wpool.tile([P, CC, C], FP32R)
        wthetaT = wpool.tile([P, CC, C], FP32R)
        wmT = wpool.tile([P, CC, C], FP32R)
        for w_d, wT, scale in (
            (wphi_d, wphiT, None),
            (wtheta_d, wthetaT, None),
            (wmask_d, wmT, gamma_t),  # fold gamma into the mask projection
        ):
            wnat = scratch.tile([P, CC, C], FP32, tag="s8")
            nc.sync.dma_start(out=wnat, in_=w_d.rearrange("(cc p) j -> p cc j", p=P))
            for jc in range(CC):
                pt = ps.tile([P, C], FP32, tag="ps")
                for dc in range(CC):
                    nc.tensor.transpose(
                        pt[:, dc * P:(dc + 1) * P],
                        wnat[:, dc, jc * P:(jc + 1) * P],
                        identity,
                    )
                if scale is None:
                    nc.any.tensor_copy(wT[:, jc, :], pt)
                else:
                    nc.any.tensor_scalar_mul(wT[:, jc, :], pt, scale)

        # ---- load X ([p, cc, n] <=> X[128*cc+p, n]), per-chunk DMAs
        x_sb = xpool.tile([P, CC, N], FP32R)
        for cc in range(CC):
            nc.sync.dma_start(
                out=x_sb[:, cc, :], in_=x_d[cc * P:(cc + 1) * P, :]
            )

        # ---- X^T via PE transposes: XT[p, t, c] = X[c, 128*t + p]
        xt_sb = bigpool.tile([P, NT, C], FP32R, tag="big")
        for cc in range(CC):
            for tg in range(NT // 4):
                pt = ps.tile([P, C], FP32, tag="ps")
                for k in range(4):
                    t = tg * 4 + k
                    nc.tensor.transpose(
                        pt[:, k * P:(k + 1) * P],
                        _f32(x_sb[:, cc, t * P:(t + 1) * P]),
                        identity,
                    )
                nc.any.tensor_copy(
                    xt_sb[:, tg * 4:(tg + 1) * 4, cc * P:(cc + 1) * P],
                    pt.rearrange("p (k c) -> p k c", k=4),
                )

        # ---- Gram: G[a, b] = sum_n X[a, n] X[b, n]   (symmetric)
        g_sb = scratch.tile([P, CC, C], FP32R, tag="s8")
        for mc in range(CC):
            gp = ps.tile([P, C], FP32, tag="ps")
            for t in range(NT):
                nc.tensor.matmul(
                    gp,
                    xt_sb[:, t, mc * P:(mc + 1) * P],
                    xt_sb[:, t, :],
                    start=(t == 0),
                    stop=(t == NT - 1),
                )
            nc.any.tensor_copy(g_sb[:, mc, :], gp)

        # ---- T1 = G @ Wt^T  (uses G symmetry for the stationary operand)
        t1_sb = scratch.tile([P, CC, C], FP32R, tag="s8")
        for mc in range(CC):
            tp = ps.tile([P, C], FP32, tag="ps")
            for jc in range(CC):
                nc.tensor.matmul(
                    tp,
                    g_sb[:, jc, mc * P:(mc + 1) * P],
                    wthetaT[:, jc, :],
                    start=(jc == 0),
                    stop=(jc == CC - 1),
                )
            nc.any.tensor_copy(t1_sb[:, mc, :], tp)

        # ---- L = Wp @ T1 ; softmax rows -> attn
        attn_sb = scratch.tile([P, CC, C], FP32R, tag="s8")
        for mc in range(CC):
            lp = ps.tile([P, C], FP32, tag="ps")
            for ic in range(CC):
                nc.tensor.matmul(
                    lp,
                    wphiT[:, ic, mc * P:(mc + 1) * P],
                    t1_sb[:, ic, :],
                    start=(ic == 0),
                    stop=(ic == CC - 1),
                )
            neg_max = vecs.tile([P, 1], FP32)
            nc.vector.tensor_reduce(
                out=neg_max, in_=lp, axis=mybir.AxisListType.X,
                op=mybir.AluOpType.max, negate=True,
            )
            sums = vecs.tile([P, 1], FP32)
            nc.scalar.activation(
                out=attn_sb[:, mc, :], in_=lp,
                func=mybir.ActivationFunctionType.Exp,
                bias=neg_max, scale=1.0, accum_out=sums,
            )
            rinv = vecs.tile([P, 1], FP32)
            nc.vector.reciprocal(rinv, sums)
            nc.vector.tensor_scalar_mul(
                attn_sb[:, mc, :], attn_sb[:, mc, :], rinv
            )

        # ---- attn^T via PE transposes
        attnT_sb = scratch.tile([P, CC, C], FP32R, tag="s8")
        for dc in range(CC):
            pt = ps.tile([P, C], FP32, tag="ps")
            for mc in range(CC):
                nc.tensor.transpose(
                    pt[:, mc * P:(mc + 1) * P],
                    _f32(attn_sb[:, mc, dc * P:(dc + 1) * P]),
                    identity,
                )
            nc.any.tensor_copy(attnT_sb[:, dc, :], pt)

        # ---- A'^T[j, c] = sum_d Wg[d, j] attn[c, d]
        apT_sb = scratch.tile([P, CC, C], FP32R, tag="s8")
        for jc in range(CC):
            ap_ps = ps.tile([P, C], FP32, tag="ps")
            for dc in range(CC):
                nc.tensor.matmul(
                    ap_ps,
                    wg_sb[:, dc, jc * P:(jc + 1) * P],
                    attnT_sb[:, dc, :],
                    start=(dc == 0),
                    stop=(dc == CC - 1),
                )
            nc.any.tensor_copy(apT_sb[:, jc, :], ap_ps)

        # ---- y^T blocks straight into Z layout.
        # Z[i, q*512 + r] = y^T[8*i + q, r]; with n = 1024*ci + 8*m + q the
        # output PSUM tile [m, r] equals ZS[:, ci, q*512:(q+1)*512].
        zs_sb = bigpool.tile([P, CC, N], FP32R, tag="big")
        for ci in range(CC):
            for q in range(QF):
                zp = ps.tile([P, C], FP32, tag="ps")
                for jc in range(CC):
                    xr = x_sb[:, jc, :].rearrange(
                        "p (ci m q) -> p ci q m", ci=CC, q=QF
                    )
                    nc.tensor.matmul(
                        zp,
                        xr[:, ci, q, :],
                        apT_sb[:, jc, :],
                        start=(jc == 0),
                        stop=(jc == CC - 1),
                    )
                nc.any.tensor_copy(zs_sb[:, ci, q * C:(q + 1) * C], zp)

        # ---- out = (gamma*Wm) @ Z + x
        for oc in range(CC):
            for jb in range(N // C):
                mp = ps.tile([P, C], FP32, tag="ps")
                for ic in range(CC):
                    nc.tensor.matmul(
                        mp,
                        wmT[:, ic, oc * P:(oc + 1) * P],
                        zs_sb[:, ic, jb * C:(jb + 1) * C],
                        start=(ic == 0),
                        stop=(ic == CC - 1),
                    )
                ot = outp.tile([P, C], FP32)
                nc.vector.tensor_add(
                    ot, mp, _f32(x_sb[:, oc, jb * C:(jb + 1) * C])
                )
                nc.sync.dma_start(
                    out=out_d[oc * P:(oc + 1) * P, jb * C:(jb + 1) * C], in_=ot
                )


_NC_CACHE = {}
LAST_RESULT = None


def get_nc():
    if "nc" not in _NC_CACHE:
        _NC_CACHE["nc"] = _build_nc()
    return _NC_CACHE["nc"]


def _round_fp32r(x):
    """Round fp32 array to the fp32r grid (11 explicit mantissa bits, RNE)."""
    u = np.ascontiguousarray(x, dtype=np.float32).view(np.uint32).astype(np.uint64)
    shift = 23 - 11
    add = (np.uint64(1) << np.uint64(shift - 1)) - np.uint64(1) + (
        (u >> np.uint64(shift)) & np.uint64(1)
    )
    u = (u + add) & np.uint64(~((1 << shift) - 1) & 0xFFFFFFFF)
    return u.astype(np.uint32).view(np.float32)


def kernel(x, w_phi, w_theta, w_g, w_mask, gamma):
    global LAST_RESULT
    x = np.ascontiguousarray(np.asarray(x, dtype=np.float32))
    w_phi = _round_fp32r(np.asarray(w_phi, dtype=np.float32))
    w_theta = _round_fp32r(np.asarray(w_theta, dtype=np.float32))
    w_g = _round_fp32r(np.asarray(w_g, dtype=np.float32))
    w_mask = _round_fp32r(np.asarray(w_mask, dtype=np.float32))
    gamma = np.ascontiguousarray(np.asarray(gamma, dtype=np.float32))

    B, c, h, w = x.shape
    assert (c, h * w) == (C, N), (x.shape,)
    nc = get_nc()

    in_maps = [
        {
            "x": _round_fp32r(x[b].reshape(C, N)),
            "w_phi": w_phi,
            "w_theta": w_theta,
            "w_g": w_g,
            "w_mask": w_mask,
            "gamma": gamma,
        }
        for b in range(B)
    ]
    trace = bool(int(os.environ.get("KERNEL_TRACE", "0")))
    res = run_bass_kernel_spmd(nc, in_maps, list(range(B)), trace=trace)
    LAST_RESULT = res
    out = np.stack([res.results[b]["out"].reshape(c, h, w) for b in range(B)])
    return out


# revision 7
# speedup vs baseline: 1.0157x; 1.0157x over previous
# Trainium2 Bass kernel for per-sample channel-attention module (CAM).
#
# Reference math per sample (C=512, N=H*W=4096):
#   X = x.reshape(C, N)
#   phi = Wp X ; theta = Wt X ; g = Wg X
#   attn = softmax_rows(phi @ theta^T)          # [C, C]
#   y = attn @ g                                 # [C, N]
#   Z = (y^T).flatten().reshape(C, N)            # torch permute+view reinterpretation
#   out = gamma * (Wm @ Z) + x
#
# Algebraic restructuring used here (cuts PE work ~1.8x):
#   G = X X^T                  (Gram, [C, C])
#   L = Wp G Wt^T              (attention logits via two small GEMMs)
#   A' = softmax(L) @ Wg       (fold g-projection into attention)
#   y = A' X                   (single big GEMM)
# The Z reinterpretation is handled for free by computing y^T blocks with a
# stride-8 column selection of X as the stationary matmul operand, so each
# PSUM tile lands exactly on a contiguous block of Z's SBUF layout.
#
# All matmuls/transposes run in float32r (fp32 data, PE streams it at
# 1 row/cycle; values carry 11 explicit mantissa bits). The BIR verifier
# requires every fp32r matmul operand to be produced as fp32r, so operand
# tiles are allocated float32r and the host pre-rounds the input tensors
# to the fp32r grid (round-to-nearest-even, 11 mantissa bits).

import os
import numpy as np

import concourse.bass as bass
import concourse.mybir as mybir
import concourse.tile as tile
from concourse import bacc
from concourse.bass_utils import run_bass_kernel_spmd
from concourse.tile import TileContext
from concourse.masks import make_identity

P = 128          # partitions
C = 512          # channels
N = 4096         # spatial (64*64)
CC = C // P      # 4 channel chunks
NT = N // P      # 32 spatial tiles
QF = N // C      # 8 fold factor for the permute+view reinterpretation
FP32 = mybir.dt.float32
FP32R = mybir.dt.float32r


def _f32(ap):
    # reinterpret an fp32r tile as plain fp32 (identical bit layout)
    return ap.bitcast(FP32)


def _build_nc():
    nc = bacc.Bacc("TRN2", target_bir_lowering=False, debug=False, num_devices=8)
    # x is consumed both as a matmul operand and in the residual add, so it
    # is declared fp32r and pre-rounded host-side (residual error ~2^-13).
    x_d = nc.dram_tensor("x", [C, N], FP32R, kind="ExternalInput").ap()
    wphi_d = nc.dram_tensor("w_phi", [C, C], FP32, kind="ExternalInput").ap()
    wtheta_d = nc.dram_tensor("w_theta", [C, C], FP32, kind="ExternalInput").ap()
    wg_d = nc.dram_tensor("w_g", [C, C], FP32R, kind="ExternalInput").ap()
    wmask_d = nc.dram_tensor("w_mask", [C, C], FP32, kind="ExternalInput").ap()
    gamma_d = nc.dram_tensor("gamma", [1], FP32, kind="ExternalInput").ap()
    out_d = nc.dram_tensor("out", [C, N], FP32, kind="ExternalOutput").ap()

    with TileContext(nc) as tc:
        _body(tc, x_d, wphi_d, wtheta_d, wg_d, wmask_d, gamma_d, out_d)
    nc.compile()
    return nc


def _body(tc, x_d, wphi_d, wtheta_d, wg_d, wmask_d, gamma_d, out_d):
    nc = tc.nc
    from contextlib import ExitStack

    with ExitStack() as ctx:
        const = ctx.enter_context(tc.tile_pool(name="const", bufs=1))
        xpool = ctx.enter_context(tc.tile_pool(name="xpool", bufs=1))
        wpool = ctx.enter_context(tc.tile_pool(name="wpool", bufs=1))
        bigpool = ctx.enter_context(tc.tile_pool(name="bigpool", bufs=1))
        scratch = ctx.enter_context(tc.tile_pool(name="scratch", bufs=2))
        vecs = ctx.enter_context(tc.tile_pool(name="vecs", bufs=8))
        outp = ctx.enter_context(tc.tile_pool(name="outp", bufs=4))
        ps = ctx.enter_context(tc.tile_pool(name="ps", bufs=4, space="PSUM"))
        psg = ctx.enter_context(tc.tile_pool(name="psg", bufs=4, space="PSUM"))

        # Transposes run in plain fp32 (exact); the PSUM->SBUF copy after
        # each transpose performs the fp32r rounding for matmul operands.
        identity = const.tile([P, P], FP32)
        make_identity(nc, identity)

        gamma_t = const.tile([P, 1], FP32)
        nc.gpsimd.dma_start(out=gamma_t, in_=gamma_d.to_broadcast([P, 1]))

        # ---- load weights (natural layout, [p, cc, j] <=> W[128*cc+p, j]) and
        # build transposed copies WT[p, jc, d] = W[d, 128*jc + p].
        wg_sb = wpool.tile([P, CC, C], FP32R)
        nc.sync.dma_start(out=wg_sb, in_=wg_d.rearrange("(cc p) j -> p cc j", p=P))

        wphiT = wpool.tile([P, CC, C], FP32R)
        wthetaT = wpool.tile([P, CC, C], FP32R)
        wmT = wpool.tile([P, CC, C], FP32R)
        for w_d, wT, scale in (
            (wphi_d, wphiT, None),
            (wtheta_d, wthetaT, None),
            (wmask_d, wmT, gamma_t),  # fold gamma into the mask projection
        ):
            wnat = scratch.tile([P, CC, C], FP32, tag="s8")
            nc.sync.dma_start(out=wnat, in_=w_d.rearrange("(cc p) j -> p cc j", p=P))
            for jc in range(CC):
                pt = ps.tile([P, C], FP32, tag="ps")
                for dc in range(CC):
                    nc.tensor.transpose(
                        pt[:, dc * P:(dc + 1) * P],
                        wnat[:, dc, jc * P:(jc + 1) * P],
                        identity,
                    )
                if scale is None:
                    nc.any.tensor_copy(wT[:, jc, :], pt)
                else:
                    nc.any.tensor_scalar_mul(wT[:, jc, :], pt, scale)

        # ---- streamed X load + transpose + Gram accumulation.
        # X arrives as 32 column blocks [512, 128]; each is transposed on PE
        # into XT[p, t, c] = X[c, 128*t + p] and immediately folded into the
        # four Gram accumulators, so DMA, transposes, and Gram matmuls all
        # overlap from the first 256 KB onward.
        x_sb = xpool.tile([P, CC, N], FP32R)
        xt_sb = bigpool.tile([P, NT, C], FP32R, tag="big")
        gacc = [
            psg.tile([P, C], FP32, tag="gacc", name=f"gacc{i}")
            for i in range(CC)
        ]
        for t in range(NT):
            nc.sync.dma_start(
                out=x_sb[:, :, t * P:(t + 1) * P],
                in_=x_d[:, t * P:(t + 1) * P].rearrange(
                    "(cc p) n -> p cc n", p=P
                ),
            )
            pt = ps.tile([P, C], FP32, tag="ps")
            for cc in range(CC):
                nc.tensor.transpose(
                    pt[:, cc * P:(cc + 1) * P],
                    _f32(x_sb[:, cc, t * P:(t + 1) * P]),
                    identity,
                )
            nc.any.tensor_copy(xt_sb[:, t, :], pt)
            for mc in range(CC):
                nc.tensor.matmul(
                    gacc[mc],
                    xt_sb[:, t, mc * P:(mc + 1) * P],
                    xt_sb[:, t, :],
                    start=(t == 0),
                    stop=(t == NT - 1),
                )

        # ---- Gram result to SBUF
        g_sb = scratch.tile([P, CC, C], FP32R, tag="s8")
        for mc in range(CC):
            nc.any.tensor_copy(g_sb[:, mc, :], gacc[mc])

        # ---- T1 = G @ Wt^T  (uses G symmetry for the stationary operand)
        t1_sb = scratch.tile([P, CC, C], FP32R, tag="s8")
        for mc in range(CC):
            tp = ps.tile([P, C], FP32, tag="ps")
            for jc in range(CC):
                nc.tensor.matmul(
                    tp,
                    g_sb[:, jc, mc * P:(mc + 1) * P],
                    wthetaT[:, jc, :],
                    start=(jc == 0),
                    stop=(jc == CC - 1),
                )
            nc.any.tensor_copy(t1_sb[:, mc, :], tp)

        # ---- L = Wp @ T1 ; softmax rows -> attn
        attn_sb = scratch.tile([P, CC, C], FP32R, tag="s8")
        for mc in range(CC):
            lp = ps.tile([P, C], FP32, tag="ps")
            for ic in range(CC):
                nc.tensor.matmul(
                    lp,
                    wphiT[:, ic, mc * P:(mc + 1) * P],
                    t1_sb[:, ic, :],
                    start=(ic == 0),
                    stop=(ic == CC - 1),
                )
            neg_max = vecs.tile([P, 1], FP32)
            nc.vector.tensor_reduce(
                out=neg_max, in_=lp, axis=mybir.AxisListType.X,
                op=mybir.AluOpType.max, negate=True,
            )
            sums = vecs.tile([P, 1], FP32)
            nc.scalar.activation(
                out=attn_sb[:, mc, :], in_=lp,
                func=mybir.ActivationFunctionType.Exp,
                bias=neg_max, scale=1.0, accum_out=sums,
            )
            rinv = vecs.tile([P, 1], FP32)
            nc.vector.reciprocal(rinv, sums)
            nc.vector.tensor_scalar_mul(
                attn_sb[:, mc, :], attn_sb[:, mc, :], rinv
            )

        # ---- attn^T via PE transposes
        attnT_sb = scratch.tile([P, CC, C], FP32R, tag="s8")
        for dc in range(CC):
            pt = ps.tile([P, C], FP32, tag="ps")
            for mc in range(CC):
                nc.tensor.transpose(
                    pt[:, mc * P:(mc + 1) * P],
                    _f32(attn_sb[:, mc, dc * P:(dc + 1) * P]),
                    identity,
                )
            nc.any.tensor_copy(attnT_sb[:, dc, :], pt)

        # ---- A'^T[j, c] = sum_d Wg[d, j] attn[c, d]
        apT_sb = scratch.tile([P, CC, C], FP32R, tag="s8")
        for jc in range(CC):
            ap_ps = ps.tile([P, C], FP32, tag="ps")
            for dc in range(CC):
                nc.tensor.matmul(
                    ap_ps,
                    wg_sb[:, dc, jc * P:(jc + 1) * P],
                    attnT_sb[:, dc, :],
                    start=(dc == 0),
                    stop=(dc == CC - 1),
                )
            nc.any.tensor_copy(apT_sb[:, jc, :], ap_ps)

        # ---- y^T blocks straight into Z layout.
        # Z[i, q*512 + r] = y^T[8*i + q, r]; with n = 1024*ci + 8*m + q the
        # output PSUM tile [m, r] equals ZS[:, ci, q*512:(q+1)*512].
        # Interleaved q-major: the mask GEMM for output column block jb only
        # needs the four ZS blocks (ci=0..3, q=jb), so each q round runs
        # 16 ZS matmuls followed by 16 mask matmuls with epilogue overlapped.
        zs_sb = bigpool.tile([P, CC, N], FP32R, tag="big")
        for q in range(QF):
            for ci in range(CC):
                zp = ps.tile([P, C], FP32, tag="ps")
                for jc in range(CC):
                    xr = x_sb[:, jc, :].rearrange(
                        "p (ci m q) -> p ci q m", ci=CC, q=QF
                    )
                    nc.tensor.matmul(
                        zp,
                        xr[:, ci, q, :],
                        apT_sb[:, jc, :],
                        start=(jc == 0),
                        stop=(jc == CC - 1),
                    )
                nc.any.tensor_copy(zs_sb[:, ci, q * C:(q + 1) * C], zp)

            # out[:, q-block] = (gamma*Wm) @ Z[:, q-block] + x[:, q-block]
            jb = q
            for oc in range(CC):
                mp = psg.tile([P, C], FP32, tag="gacc")
                for ic in range(CC):
                    nc.tensor.matmul(
                        mp,
                        wmT[:, ic, oc * P:(oc + 1) * P],
                        zs_sb[:, ic, jb * C:(jb + 1) * C],
                        start=(ic == 0),
                        stop=(ic == CC - 1),
                    )
                ot = outp.tile([P, C], FP32)
                nc.vector.tensor_add(
                    ot, mp, _f32(x_sb[:, oc, jb * C:(jb + 1) * C])
                )
                nc.sync.dma_start(
                    out=out_d[oc * P:(oc + 1) * P, jb * C:(jb + 1) * C], in_=ot
                )


_NC_CACHE = {}
LAST_RESULT = None


def get_nc():
    if "nc" not in _NC_CACHE:
        _NC_CACHE["nc"] = _build_nc()
    return _NC_CACHE["nc"]


def _round_fp32r(x):
    """Round fp32 array to the fp32r grid (11 explicit mantissa bits, RNE)."""
    u = np.ascontiguousarray(x, dtype=np.float32).view(np.uint32).astype(np.uint64)
    shift = 23 - 11
    add = (np.uint64(1) << np.uint64(shift - 1)) - np.uint64(1) + (
        (u >> np.uint64(shift)) & np.uint64(1)
    )
    u = (u + add) & np.uint64(~((1 << shift) - 1) & 0xFFFFFFFF)
    return u.astype(np.uint32).view(np.float32)


def kernel(x, w_phi, w_theta, w_g, w_mask, gamma):
    global LAST_RESULT
    x = np.ascontiguousarray(np.asarray(x, dtype=np.float32))
    w_phi = _round_fp32r(np.asarray(w_phi, dtype=np.float32))
    w_theta = _round_fp32r(np.asarray(w_theta, dtype=np.float32))
    w_g = _round_fp32r(np.asarray(w_g, dtype=np.float32))
    w_mask = _round_fp32r(np.asarray(w_mask, dtype=np.float32))
    gamma = np.ascontiguousarray(np.asarray(gamma, dtype=np.float32))

    B, c, h, w = x.shape
    assert (c, h * w) == (C, N), (x.shape,)
    nc = get_nc()

    in_maps = [
        {
            "x": _round_fp32r(x[b].reshape(C, N)),
            "w_phi": w_phi,
            "w_theta": w_theta,
            "w_g": w_g,
            "w_mask": w_mask,
            "gamma": gamma,
        }
        for b in range(B)
    ]
    trace = bool(int(os.environ.get("KERNEL_TRACE", "0")))
    res = run_bass_kernel_spmd(nc, in_maps, list(range(B)), trace=trace)
    LAST_RESULT = res
    out = np.stack([res.results[b]["out"].reshape(c, h, w) for b in range(B)])
    return out
